# revision 12
# baseline (speedup 1.0000x reference)
"""Distributed Trainium2 kernel for nn_AttnDecoderLSTM.

Sharding (8 cores):
  - Attention: data-parallel over batch B=64 (8 batches/core). Each core
    computes its score columns from its encoder_outputs shard, AllGathers the
    tiny [64,256] score matrix, replicates the (cheap, quirky reshape-)softmax,
    then computes attention sums for its local batches via PE matvecs.
  - LSTM: tensor-parallel over hidden (64 hidden/core, all 4 gates).  x and h
    are AllGathered in transposed [feature, batch] layout, which is exactly the
    lhsT layout the next matmul needs.
  - Output projection: tensor-parallel over vocab (4000/core), in bf16.

Math note: the reference adds h_top @ Wh.T + attn_b to the scores, but both
terms are constant within each 64-wide softmax chunk (the chunk spans one
batch row's consecutive columns), so they cancel in the softmax exactly and
are omitted.
"""

import os
import sys

import numpy as np

sys.path.insert(0, "/opt/trn_rl_repo")

import concourse.bass as bass
import concourse.bacc as bacc
import concourse.mybir as mybir
from concourse import tile
from concourse.bass_utils import run_bass_kernel_spmd

F32 = mybir.dt.float32
BF16 = mybir.dt.bfloat16
AX = mybir.AxisListType.X
AFT = mybir.ActivationFunctionType
ALU = mybir.AluOpType

NCORES = 8
B, H, S, V = 64, 512, 256, 32000
BL = B // NCORES      # 8 batches per core
JL = H // NCORES      # 64 hidden per core
VL = V // NCORES      # 4000 vocab per core
RG = [list(range(NCORES))]

_NC = None
LAST_RESULTS = None


def _build():
    nc = bacc.Bacc()

    def pi(name, shape, dt=F32):
        return nc.declare_dram_parameter(name, list(shape), dt, isOutput=False)

    def po(name, shape, dt=F32):
        return nc.declare_dram_parameter(name, list(shape), dt, isOutput=True)

    enc_p = pi("enc", [2, 128, BL * 1024])       # [s-tile, s, (b d)]
    web_p = pi("web", [128, 1024])               # We bcast over partitions
    sel_p = pi("sel", [128, BL, 64])             # one-hot b_local selectors
    idn_p = pi("idn", [128, 128])                # identity for PE transpose
    inT_p = pi("inT", [4, 128, B])               # input.T  (k-tiles)
    h0T0_p = pi("h0T0", [4, 128, B])
    h0T1_p = pi("h0T1", [4, 128, B])
    c0l0_p = pi("c0l0", [B, JL])
    c0l1_p = pi("c0l1", [B, JL])
    w0T_p = pi("w0T", [12, 128, 4 * JL])         # w_ih0 shard, transposed
    whh0T_p = pi("whh0T", [4, 128, 4 * JL])
    b0_p = pi("b0", [B, 4 * JL])
    w1T_p = pi("w1T", [4, 128, 4 * JL])
    whh1T_p = pi("whh1T", [4, 128, 4 * JL])
    b1_p = pi("b1", [B, 4 * JL])
    owT_p = pi("owT", [4, 128, VL], BF16)        # out_W shard, transposed, bf16
    ob_p = pi("ob", [B, VL], BF16)

    h1_o = po("h1_l", [B, JL])
    c1_o = po("c1_l", [B, JL])
    h2_o = po("h2_l", [B, JL])
    c2_o = po("c2_l", [B, JL])
    pred_o = po("pred_l", [B, VL])

    with tile.TileContext(nc) as tc:
        with (
            tc.tile_pool(name="sb", bufs=1) as sb,
            tc.tile_pool(name="ps", bufs=1, space="PSUM") as ps,
            tc.tile_pool(name="ps2", bufs=2, space="PSUM") as ps2,
            tc.tile_pool(name="dram", bufs=1, space="DRAM") as dram,
        ):
            # ---------------- loads ----------------
            enc_sb = []
            for t in range(2):
                e = sb.tile([128, BL, 1024], F32, tag=f"enc{t}")
                nc.sync.dma_start(e[:], enc_p[t].rearrange("p (b d) -> p b d", b=BL))
                enc_sb.append(e)

            web_sb = sb.tile([128, 1024], F32, tag="web")
            nc.sync.dma_start(web_sb[:], web_p[:])
            sel_sb = sb.tile([128, BL, 64], F32, tag="sel")
            nc.sync.dma_start(sel_sb[:], sel_p[:])
            idn_sb = sb.tile([128, 128], F32, tag="idn")
            nc.sync.dma_start(idn_sb[:], idn_p[:])

            inT_sb = sb.tile([128, 4, B], F32, tag="inT")
            nc.sync.dma_start(inT_sb[:], inT_p[:].rearrange("k p b -> p k b"))
            h0T0_sb = sb.tile([128, 4, B], F32, tag="h0T0")
            nc.sync.dma_start(h0T0_sb[:], h0T0_p[:].rearrange("k p b -> p k b"))
            h0T1_sb = sb.tile([128, 4, B], F32, tag="h0T1")
            nc.sync.dma_start(h0T1_sb[:], h0T1_p[:].rearrange("k p b -> p k b"))
            c0l0_sb = sb.tile([B, JL], F32, tag="c0l0")
            nc.sync.dma_start(c0l0_sb[:], c0l0_p[:])
            c0l1_sb = sb.tile([B, JL], F32, tag="c0l1")
            nc.sync.dma_start(c0l1_sb[:], c0l1_p[:])

            w0T_sb = sb.tile([128, 12, 4 * JL], F32, tag="w0T")
            nc.sync.dma_start(w0T_sb[:], w0T_p[:].rearrange("k p j -> p k j"))
            whh0T_sb = sb.tile([128, 4, 4 * JL], F32, tag="whh0T")
            nc.sync.dma_start(whh0T_sb[:], whh0T_p[:].rearrange("k p j -> p k j"))
            b0_sb = sb.tile([B, 4 * JL], F32, tag="b0")
            nc.sync.dma_start(b0_sb[:], b0_p[:])
            w1T_sb = sb.tile([128, 4, 4 * JL], F32, tag="w1T")
            nc.sync.dma_start(w1T_sb[:], w1T_p[:].rearrange("k p j -> p k j"))
            whh1T_sb = sb.tile([128, 4, 4 * JL], F32, tag="whh1T")
            nc.sync.dma_start(whh1T_sb[:], whh1T_p[:].rearrange("k p j -> p k j"))
            b1_sb = sb.tile([B, 4 * JL], F32, tag="b1")
            nc.sync.dma_start(b1_sb[:], b1_p[:])

            owT_sb = sb.tile([128, 4, VL], BF16, tag="owT")
            for k in range(4):
                nc.sync.dma_start(owT_sb[:, k, :], owT_p[k])
            ob_sb = sb.tile([B, VL], BF16, tag="ob")
            nc.sync.dma_start(ob_sb[:], ob_p[:])

            # ------------- scores: es[s, b] = enc[s,b,:] . We -------------
            prod = sb.tile([128, 1024], F32, tag="prod")
            es_sb = sb.tile([128, 2, BL], F32, tag="es")
            for t in range(2):
                for b in range(BL):
                    nc.vector.tensor_mul(prod[:], enc_sb[t][:, b, :], web_sb[:])
                    nc.vector.reduce_sum(
                        out=es_sb[:, t, b : b + 1], in_=prod[:], axis=AX
                    )

            sc_in = dram.tile([BL, 256], F32, tag="sc_in")
            sc_out = dram.tile([B, 256], F32, tag="sc_out", addr_space="Shared")
            # write scores b-major: element (p,t,b) -> addr b*256 + t*128 + p
            for t in range(2):
                nc.gpsimd.dma_start(
                    sc_in[:].rearrange("b (t p) -> p t b", t=2)[:, t, :],
                    es_sb[:, t, :],
                )
            nc.gpsimd.collective_compute(
                "AllGather",
                ALU.bypass,
                ins=[sc_in.opt()],
                outs=[sc_out.opt()],
                replica_groups=RG,
            )

            # ------------- softmax over 64-wide chunks -------------
            sc_sb = sb.tile([B, 4, 64], F32, tag="sc")
            nc.sync.dma_start(sc_sb[:], sc_out[:].rearrange("q (r b) -> q r b", r=4))
            m_sb = sb.tile([B, 4], F32, tag="m")
            for r in range(4):
                nc.vector.reduce_max(
                    out=m_sb[:, r : r + 1], in_=sc_sb[:, r, :], axis=AX
                )
            negm = sb.tile([B, 4], F32, tag="negm")
            nc.scalar.mul(negm[:], m_sb[:], -1.0)
            e_sb = sb.tile([B, 4, 64], F32, tag="e")
            d_sb = sb.tile([B, 4], F32, tag="d")
            for r in range(4):
                nc.scalar.activation(
                    e_sb[:, r, :],
                    sc_sb[:, r, :],
                    AFT.Exp,
                    bias=negm[:, r : r + 1],
                    accum_out=d_sb[:, r : r + 1],
                )
            rd_sb = sb.tile([B, 4], F32, tag="rd")
            nc.vector.reciprocal(rd_sb[:], d_sb[:])
            n_sb = sb.tile([B, 4, 64], F32, tag="n")
            for r in range(4):
                nc.vector.tensor_scalar_mul(
                    n_sb[:, r, :], e_sb[:, r, :], rd_sb[:, r : r + 1]
                )

            # remap n[q, r, b] -> w[s, b] with s = 4q + r  (via DRAM)
            wrem = dram.tile([256, B], F32, tag="wrem")
            nc.gpsimd.dma_start(
                wrem[:].rearrange("(q r) b -> q r b", r=4), n_sb[:]
            )
            w_sb = sb.tile([128, 2, B], F32, tag="wsb")
            nc.gpsimd.dma_start(
                w_sb[:], wrem[:].rearrange("(t p) b -> p t b", t=2)
            )

            # select local batch columns: wse[s, i] = sum_b w[s, b] sel[i, b]
            wse = sb.tile([128, 2, BL], F32, tag="wse")
            wprod = sb.tile([128, 64], F32, tag="wprod")
            for t in range(2):
                for i in range(BL):
                    nc.vector.tensor_mul(wprod[:], w_sb[:, t, :], sel_sb[:, i, :])
                    nc.vector.reduce_sum(
                        out=wse[:, t, i : i + 1], in_=wprod[:], axis=AX
                    )

            # ------------- attention sums (per local batch matvec) -------------
            # at2_ps[d, dc, b] = sum_s enc[s, b, dc*128+d] * w[s, b]
            # (enc tile is the stationary operand, weight column the moving one)
            at_ps = ps.tile([128, 8, BL], F32, tag="atp")
            for b in range(BL):
                for dc in range(8):
                    for t in range(2):
                        nc.tensor.matmul(
                            at_ps[:, dc, b : b + 1],
                            enc_sb[t][:, b, dc * 128 : (dc + 1) * 128],
                            wse[:, t, b : b + 1],
                            start=(t == 0),
                            stop=(t == 1),
                        )
            at_sb = sb.tile([128, 8, BL], F32, tag="atsb")
            nc.vector.tensor_copy(at_sb[:], at_ps[:])
            at_in = dram.tile([BL, 1024], F32, tag="at_in")
            at_out = dram.tile([B, 1024], F32, tag="at_out", addr_space="Shared")
            # DMA does the [d, b] -> [b, d] transpose: addr = b*1024 + dc*128 + p
            at_in_r = at_in[:].rearrange("b (dc p) -> p dc b", dc=8)
            for dc in range(8):
                nc.sync.dma_start(at_in_r[:, dc, :], at_sb[:, dc, :])
            nc.gpsimd.collective_compute(
                "AllGather",
                ALU.bypass,
                ins=[at_in.opt()],
                outs=[at_out.opt()],
                replica_groups=RG,
            )

            # ------------- x^T: transpose attn_full, concat input^T -------------
            af_sb = sb.tile([B, 1024], F32, tag="af")
            nc.sync.dma_start(af_sb[:], at_out[:])
            xT_sb = sb.tile([128, 8, B], F32, tag="xT")
            for c in range(8):
                tp = ps.tile([128, B], F32, tag="tp")
                nc.tensor.matmul(
                    tp[:],
                    af_sb[:, c * 128 : (c + 1) * 128],
                    idn_sb[:B, :B],
                    is_transpose=True,
                )
                nc.vector.tensor_copy(xT_sb[:, c, :], tp[:])

            # ------------- LSTM layer 0 (gates for all B, local hidden) ---------
            g0_ps = ps.tile([B, 4 * JL], F32, tag="g")
            nmm = 16
            k = 0
            for c in range(8):
                nc.tensor.matmul(
                    g0_ps[:], xT_sb[:, c, :], w0T_sb[:, c, :],
                    start=(k == 0), stop=(k == nmm - 1),
                )
                k += 1
            for c in range(4):
                nc.tensor.matmul(
                    g0_ps[:], inT_sb[:, c, :], w0T_sb[:, 8 + c, :],
                    start=(k == 0), stop=(k == nmm - 1),
                )
                k += 1
            for c in range(4):
                nc.tensor.matmul(
                    g0_ps[:], h0T0_sb[:, c, :], whh0T_sb[:, c, :],
                    start=(k == 0), stop=(k == nmm - 1),
                )
                k += 1

            def lstm_cell(g_ps, b_sb, c0_sb, c_out, h_out, name):
                g_sb = sb.tile([B, 4 * JL], F32, tag=f"g{name}")
                nc.vector.tensor_add(g_sb[:], g_ps[:], b_sb[:])
                act = sb.tile([B, 4, JL], F32, tag=f"act{name}")
                nc.scalar.activation(act[:, 0, :], g_sb[:, 0:64], AFT.Sigmoid)
                nc.scalar.activation(act[:, 1, :], g_sb[:, 64:128], AFT.Sigmoid)
                nc.scalar.activation(act[:, 2, :], g_sb[:, 128:192], AFT.Tanh)
                nc.scalar.activation(act[:, 3, :], g_sb[:, 192:256], AFT.Sigmoid)
                t1 = sb.tile([B, JL], F32, tag=f"t1{name}")
                nc.vector.tensor_mul(t1[:], act[:, 1, :], c0_sb[:])
                t2 = sb.tile([B, JL], F32, tag=f"t2{name}")
                nc.vector.tensor_mul(t2[:], act[:, 0, :], act[:, 2, :])
                c_sb = sb.tile([B, JL], F32, tag=f"c{name}")
                nc.vector.tensor_add(c_sb[:], t1[:], t2[:])
                nc.sync.dma_start(c_out[:], c_sb[:])
                tc1 = sb.tile([B, JL], F32, tag=f"tc{name}")
                nc.scalar.activation(tc1[:], c_sb[:], AFT.Tanh)
                h_sb = sb.tile([B, JL], F32, tag=f"h{name}")
                nc.vector.tensor_mul(h_sb[:], act[:, 3, :], tc1[:])
                nc.sync.dma_start(h_out[:], h_sb[:])
                return h_sb

            h1_sb = lstm_cell(g0_ps, b0_sb, c0l0_sb, c1_o, h1_o, "0")

            # transpose h1_l -> [JL, B], AllGather -> h1T_full [H, B]
            h1t_ps = ps.tile([JL, B], F32, tag="ht")
            nc.tensor.matmul(h1t_ps[:], h1_sb[:], idn_sb[:B, :B], is_transpose=True)
            h1t_sb = sb.tile([JL, B], F32, tag="h1ts")
            nc.vector.tensor_copy(h1t_sb[:], h1t_ps[:])
            h1_in = dram.tile([JL, B], F32, tag="h1_in")
            h1_out = dram.tile([H, B], F32, tag="h1_out", addr_space="Shared")
            nc.sync.dma_start(h1_in[:], h1t_sb[:])
            nc.gpsimd.collective_compute(
                "AllGather", ALU.bypass,
                ins=[h1_in.opt()], outs=[h1_out.opt()], replica_groups=RG,
            )
            h1T_sb = sb.tile([128, 4, B], F32, tag="h1T")
            nc.sync.dma_start(
                h1T_sb[:], h1_out[:].rearrange("(k p) b -> p k b", k=4)
            )

            # ------------- LSTM layer 1 -------------
            g1_ps = ps.tile([B, 4 * JL], F32, tag="g")
            nmm = 8
            k = 0
            for c in range(4):
                nc.tensor.matmul(
                    g1_ps[:], h1T_sb[:, c, :], w1T_sb[:, c, :],
                    start=(k == 0), stop=(k == nmm - 1),
                )
                k += 1
            for c in range(4):
                nc.tensor.matmul(
                    g1_ps[:], h0T1_sb[:, c, :], whh1T_sb[:, c, :],
                    start=(k == 0), stop=(k == nmm - 1),
                )
                k += 1

            h2_sb = lstm_cell(g1_ps, b1_sb, c0l1_sb, c2_o, h2_o, "1")

            h2t_ps = ps.tile([JL, B], F32, tag="ht")
            nc.tensor.matmul(h2t_ps[:], h2_sb[:], idn_sb[:B, :B], is_transpose=True)
            h2t_sb = sb.tile([JL, B], F32, tag="h2ts")
            nc.vector.tensor_copy(h2t_sb[:], h2t_ps[:])
            h2_in = dram.tile([JL, B], F32, tag="h2_in")
            h2_out = dram.tile([H, B], F32, tag="h2_out", addr_space="Shared")
            nc.sync.dma_start(h2_in[:], h2t_sb[:])
            nc.gpsimd.collective_compute(
                "AllGather", ALU.bypass,
                ins=[h2_in.opt()], outs=[h2_out.opt()], replica_groups=RG,
            )
            h2T_sb = sb.tile([128, 4, B], BF16, tag="h2T")
            nc.gpsimd.dma_start(
                h2T_sb[:], h2_out[:].rearrange("(k p) b -> p k b", k=4)
            )

            # ------------- vocab projection (bf16, local 4000 cols) -------------
            NB = 8
            CH = VL // NB  # 500 (one matmul must fit one PSUM bank)
            for nb in range(NB):
                pp = ps2.tile([B, CH], F32, tag="pp")
                for k in range(4):
                    nc.tensor.matmul(
                        pp[:],
                        h2T_sb[:, k, :],
                        owT_sb[:, k, nb * CH : (nb + 1) * CH],
                        start=(k == 0),
                        stop=(k == 3),
                    )
                pr = sb.tile([B, CH], F32, tag="pr")
                nc.vector.tensor_add(pr[:], pp[:], ob_sb[:, nb * CH : (nb + 1) * CH])
                nc.sync.dma_start(pred_o[:, nb * CH : (nb + 1) * CH], pr[:])

    if not nc.is_finalized():
        nc.finalize()
    return nc


def _prep_inputs(input, h0, c0, encoder_outputs, attn_W, attn_b,
                 w_ih0, w_hh0, b_ih0, b_hh0, w_ih1, w_hh1, b_ih1, b_hh1,
                 out_W, out_b):
    import ml_dtypes

    f = np.float32
    we = np.ascontiguousarray(attn_W[0, H:]).astype(f)           # [1024]
    web = np.ascontiguousarray(np.broadcast_to(we, (128, 1024)))
    idn = np.eye(128, dtype=f)
    inT = np.ascontiguousarray(input.T).reshape(4, 128, B).astype(f)
    h0T0 = np.ascontiguousarray(h0[0].T).reshape(4, 128, B).astype(f)
    h0T1 = np.ascontiguousarray(h0[1].T).reshape(4, 128, B).astype(f)

    def gate_shard(w, jsl, kdim):
        # rows grouped [i|f|g|o] x JL for the shard, then transposed -> [K, 4*JL]
        blk = w.reshape(4, H, kdim)[:, jsl, :]          # [4, JL, K]
        t = np.ascontiguousarray(np.transpose(blk, (2, 0, 1)))  # [K, 4, JL]
        return t.reshape(kdim // 128, 128, 4 * JL).astype(f)

    def bias_shard(bi, bh, jsl):
        bb = (bi + bh).reshape(4, H)[:, jsl].reshape(4 * JL)
        return np.ascontiguousarray(np.broadcast_to(bb, (B, 4 * JL))).astype(f)

    in_maps = []
    for r in range(NCORES):
        bsl = slice(r * BL, (r + 1) * BL)
        jsl = slice(r * JL, (r + 1) * JL)
        vsl = slice(r * VL, (r + 1) * VL)
        enc_r = np.ascontiguousarray(encoder_outputs[:, bsl, :]).reshape(
            2, 128, BL * 1024).astype(f)
        sel = np.zeros((BL, 64), dtype=f)
        for i in range(BL):
            sel[i, r * BL + i] = 1.0
        sel_b = np.ascontiguousarray(np.broadcast_to(sel, (128, BL, 64)))
        owT = np.ascontiguousarray(out_W[vsl].T).reshape(4, 128, VL)
        ob = np.ascontiguousarray(np.broadcast_to(out_b[vsl], (B, VL)))
        in_maps.append({
            "enc": enc_r,
            "web": web,
            "sel": sel_b,
            "idn": idn,
            "inT": inT,
            "h0T0": h0T0,
            "h0T1": h0T1,
            "c0l0": np.ascontiguousarray(c0[0][:, jsl]).astype(f),
            "c0l1": np.ascontiguousarray(c0[1][:, jsl]).astype(f),
            "w0T": gate_shard(w_ih0, jsl, 3 * H),
            "whh0T": gate_shard(w_hh0, jsl, H),
            "b0": bias_shard(b_ih0, b_hh0, jsl),
            "w1T": gate_shard(w_ih1, jsl, H),
            "whh1T": gate_shard(w_hh1, jsl, H),
            "b1": bias_shard(b_ih1, b_hh1, jsl),
            "owT": owT.astype(ml_dtypes.bfloat16),
            "ob": ob.astype(ml_dtypes.bfloat16),
        })
    return in_maps


def kernel(**inputs):
    global _NC, LAST_RESULTS
    inputs = {k: np.asarray(v) for k, v in inputs.items()}
    if _NC is None:
        _NC = _build()
    in_maps = _prep_inputs(**inputs)
    res = run_bass_kernel_spmd(
        _NC, in_maps, core_ids=list(range(NCORES)),
        trace=bool(os.environ.get("BASS_TRACE")),
    )
    LAST_RESULTS = res
    out = res.results
    h1 = np.concatenate([out[r]["h1_l"] for r in range(NCORES)], axis=1)
    c1 = np.concatenate([out[r]["c1_l"] for r in range(NCORES)], axis=1)
    h2 = np.concatenate([out[r]["h2_l"] for r in range(NCORES)], axis=1)
    c2 = np.concatenate([out[r]["c2_l"] for r in range(NCORES)], axis=1)
    pred = np.concatenate([out[r]["pred_l"] for r in range(NCORES)], axis=1)
    output = h2[None].astype(np.float32)
    h_new = np.stack([h1, h2]).astype(np.float32)
    c_new = np.stack([c1, c2]).astype(np.float32)
    return output, h_new, c_new, pred.astype(np.float32)


# revision 13
# speedup vs baseline: 1.5613x; 1.5613x over previous
"""Distributed Trainium2 kernel for nn_AttnDecoderLSTM.

Sharding (8 cores):
  - Attention: data-parallel over batch B=64 (8 batches/core). Each core
    computes its score columns from its encoder_outputs shard, AllGathers the
    tiny [64,256] score matrix, replicates the (cheap, quirky reshape-)softmax,
    then computes attention sums for its local batches via PE matvecs.
  - LSTM: tensor-parallel over hidden (64 hidden/core, all 4 gates).  x and h
    are AllGathered in transposed [feature, batch] layout, which is exactly the
    lhsT layout the next matmul needs.
  - Output projection: tensor-parallel over vocab (4000/core), in bf16.

Math note: the reference adds h_top @ Wh.T + attn_b to the scores, but both
terms are constant within each 64-wide softmax chunk (the chunk spans one
batch row's consecutive columns), so they cancel in the softmax exactly and
are omitted.
"""

import os
import sys

import numpy as np

sys.path.insert(0, "/opt/trn_rl_repo")

import concourse.bass as bass
import concourse.bacc as bacc
import concourse.mybir as mybir
from concourse import tile
from concourse.bass_utils import run_bass_kernel_spmd

F32 = mybir.dt.float32
BF16 = mybir.dt.bfloat16
AX = mybir.AxisListType.X
AFT = mybir.ActivationFunctionType
ALU = mybir.AluOpType

NCORES = 8
B, H, S, V = 64, 512, 256, 32000
BL = B // NCORES      # 8 batches per core
JL = H // NCORES      # 64 hidden per core
VL = V // NCORES      # 4000 vocab per core
RG = [list(range(NCORES))]

_NC = None
LAST_RESULTS = None


def _build():
    nc = bacc.Bacc()

    def pi(name, shape, dt=F32):
        return nc.declare_dram_parameter(name, list(shape), dt, isOutput=False)

    def po(name, shape, dt=F32):
        return nc.declare_dram_parameter(name, list(shape), dt, isOutput=True)

    enc_p = pi("enc", [2, 128, BL * 1024], BF16)  # [s-tile, s, (b d)]
    web_p = pi("web", [128, 1024], BF16)         # We bcast over partitions
    sel_p = pi("sel", [128, BL, 64])             # one-hot b_local selectors
    idn_p = pi("idn", [128, 128])                # identity for PE transpose
    inT_p = pi("inT", [4, 128, B])               # input.T  (k-tiles)
    h0T0_p = pi("h0T0", [4, 128, B])
    h0T1_p = pi("h0T1", [4, 128, B])
    c0l0_p = pi("c0l0", [B, JL])
    c0l1_p = pi("c0l1", [B, JL])
    w0T_p = pi("w0T", [12, 128, 4 * JL])         # w_ih0 shard, transposed
    whh0T_p = pi("whh0T", [4, 128, 4 * JL])
    b0_p = pi("b0", [B, 4 * JL])
    w1T_p = pi("w1T", [4, 128, 4 * JL])
    whh1T_p = pi("whh1T", [4, 128, 4 * JL])
    b1_p = pi("b1", [B, 4 * JL])
    owT_p = pi("owT", [4, 128, VL], BF16)        # out_W shard, transposed, bf16
    ob_p = pi("ob", [B, VL], BF16)

    h1_o = po("h1_l", [B, JL])
    c1_o = po("c1_l", [B, JL])
    h2_o = po("h2_l", [B, JL])
    c2_o = po("c2_l", [B, JL])
    pred_o = po("pred_l", [B, VL])

    with tile.TileContext(nc) as tc:
        with (
            tc.tile_pool(name="sb", bufs=1) as sb,
            tc.tile_pool(name="ps", bufs=1, space="PSUM") as ps,
            tc.tile_pool(name="ps2", bufs=2, space="PSUM") as ps2,
            tc.tile_pool(name="dram", bufs=1, space="DRAM") as dram,
        ):
            # warm up the collective path while input DMAs stream
            dum_in = dram.tile([BL, 8], F32, tag="dum_in")
            dum_out = dram.tile([B, 8], F32, tag="dum_out", addr_space="Shared")
            nc.gpsimd.collective_compute(
                "AllGather", ALU.bypass,
                ins=[dum_in.opt()], outs=[dum_out.opt()], replica_groups=RG,
            )

            # ---------------- loads ----------------
            enc_sb = []
            for t in range(2):
                e = sb.tile([128, BL, 1024], BF16, tag=f"enc{t}")
                nc.sync.dma_start(e[:], enc_p[t].rearrange("p (b d) -> p b d", b=BL))
                enc_sb.append(e)

            web_sb = sb.tile([128, 1024], BF16, tag="web")
            nc.sync.dma_start(web_sb[:], web_p[:])
            sel_sb = sb.tile([128, BL, 64], F32, tag="sel")
            nc.sync.dma_start(sel_sb[:], sel_p[:])
            idn_sb = sb.tile([128, 128], F32, tag="idn")
            nc.sync.dma_start(idn_sb[:], idn_p[:])

            inT_sb = sb.tile([128, 4, B], F32, tag="inT")
            nc.sync.dma_start(inT_sb[:], inT_p[:].rearrange("k p b -> p k b"))
            h0T0_sb = sb.tile([128, 4, B], F32, tag="h0T0")
            nc.sync.dma_start(h0T0_sb[:], h0T0_p[:].rearrange("k p b -> p k b"))
            h0T1_sb = sb.tile([128, 4, B], F32, tag="h0T1")
            nc.sync.dma_start(h0T1_sb[:], h0T1_p[:].rearrange("k p b -> p k b"))
            c0l0_sb = sb.tile([B, JL], F32, tag="c0l0")
            nc.sync.dma_start(c0l0_sb[:], c0l0_p[:])
            c0l1_sb = sb.tile([B, JL], F32, tag="c0l1")
            nc.sync.dma_start(c0l1_sb[:], c0l1_p[:])

            w0T_sb = sb.tile([128, 12, 4 * JL], F32, tag="w0T")
            nc.sync.dma_start(w0T_sb[:], w0T_p[:].rearrange("k p j -> p k j"))
            whh0T_sb = sb.tile([128, 4, 4 * JL], F32, tag="whh0T")
            nc.sync.dma_start(whh0T_sb[:], whh0T_p[:].rearrange("k p j -> p k j"))
            b0_sb = sb.tile([B, 4 * JL], F32, tag="b0")
            nc.sync.dma_start(b0_sb[:], b0_p[:])
            w1T_sb = sb.tile([128, 4, 4 * JL], F32, tag="w1T")
            nc.sync.dma_start(w1T_sb[:], w1T_p[:].rearrange("k p j -> p k j"))
            whh1T_sb = sb.tile([128, 4, 4 * JL], F32, tag="whh1T")
            nc.sync.dma_start(whh1T_sb[:], whh1T_p[:].rearrange("k p j -> p k j"))
            b1_sb = sb.tile([B, 4 * JL], F32, tag="b1")
            nc.sync.dma_start(b1_sb[:], b1_p[:])

            owT_sb = sb.tile([128, 4, VL], BF16, tag="owT")
            for k in range(4):
                nc.sync.dma_start(owT_sb[:, k, :], owT_p[k])
            ob_sb = sb.tile([B, VL], BF16, tag="ob")
            nc.sync.dma_start(ob_sb[:], ob_p[:])

            # ------------- scores: es[s, b] = enc[s,b,:] . We -------------
            prod = sb.tile([128, 1024], BF16, tag="prod")
            es_sb = sb.tile([128, 2, BL], F32, tag="es")
            for t in range(2):
                for b in range(BL):
                    nc.vector.tensor_mul(prod[:], enc_sb[t][:, b, :], web_sb[:])
                    nc.vector.reduce_sum(
                        out=es_sb[:, t, b : b + 1], in_=prod[:], axis=AX
                    )

            sc_in = dram.tile([BL, 256], F32, tag="sc_in")
            sc_out = dram.tile([B, 256], F32, tag="sc_out", addr_space="Shared")
            # write scores b-major: element (p,t,b) -> addr b*256 + t*128 + p
            for t in range(2):
                nc.gpsimd.dma_start(
                    sc_in[:].rearrange("b (t p) -> p t b", t=2)[:, t, :],
                    es_sb[:, t, :],
                )
            nc.gpsimd.collective_compute(
                "AllGather",
                ALU.bypass,
                ins=[sc_in.opt()],
                outs=[sc_out.opt()],
                replica_groups=RG,
            )

            # ------------- softmax over 64-wide chunks -------------
            sc_sb = sb.tile([B, 4, 64], F32, tag="sc")
            nc.sync.dma_start(sc_sb[:], sc_out[:].rearrange("q (r b) -> q r b", r=4))
            m_sb = sb.tile([B, 4], F32, tag="m")
            for r in range(4):
                nc.vector.reduce_max(
                    out=m_sb[:, r : r + 1], in_=sc_sb[:, r, :], axis=AX
                )
            negm = sb.tile([B, 4], F32, tag="negm")
            nc.scalar.mul(negm[:], m_sb[:], -1.0)
            e_sb = sb.tile([B, 4, 64], F32, tag="e")
            d_sb = sb.tile([B, 4], F32, tag="d")
            for r in range(4):
                nc.scalar.activation(
                    e_sb[:, r, :],
                    sc_sb[:, r, :],
                    AFT.Exp,
                    bias=negm[:, r : r + 1],
                    accum_out=d_sb[:, r : r + 1],
                )
            rd_sb = sb.tile([B, 4], F32, tag="rd")
            nc.vector.reciprocal(rd_sb[:], d_sb[:])
            n_sb = sb.tile([B, 4, 64], F32, tag="n")
            for r in range(4):
                nc.vector.tensor_scalar_mul(
                    n_sb[:, r, :], e_sb[:, r, :], rd_sb[:, r : r + 1]
                )

            # remap n[q, r, b] -> w[s, b] with s = 4q + r  (via DRAM)
            wrem = dram.tile([256, B], F32, tag="wrem")
            nc.gpsimd.dma_start(
                wrem[:].rearrange("(q r) b -> q r b", r=4), n_sb[:]
            )
            w_sb = sb.tile([128, 2, B], F32, tag="wsb")
            nc.gpsimd.dma_start(
                w_sb[:], wrem[:].rearrange("(t p) b -> p t b", t=2)
            )

            # select local batch columns: wse[s, i] = sum_b w[s, b] sel[i, b]
            wse = sb.tile([128, 2, BL], F32, tag="wse")
            wprod = sb.tile([128, 64], F32, tag="wprod")
            for t in range(2):
                for i in range(BL):
                    nc.vector.tensor_mul(wprod[:], w_sb[:, t, :], sel_sb[:, i, :])
                    nc.vector.reduce_sum(
                        out=wse[:, t, i : i + 1], in_=wprod[:], axis=AX
                    )

            wse_bf = sb.tile([128, 2, BL], BF16, tag="wsebf")
            nc.vector.tensor_copy(wse_bf[:], wse[:])

            # ------------- attention sums (per local batch matvec) -------------
            # at2_ps[d, dc, b] = sum_s enc[s, b, dc*128+d] * w[s, b]
            # (enc tile is the stationary operand, weight column the moving one)
            at_ps = ps.tile([128, 8, BL], F32, tag="atp")
            for b in range(BL):
                for dc in range(8):
                    for t in range(2):
                        nc.tensor.matmul(
                            at_ps[:, dc, b : b + 1],
                            enc_sb[t][:, b, dc * 128 : (dc + 1) * 128],
                            wse_bf[:, t, b : b + 1],
                            start=(t == 0),
                            stop=(t == 1),
                        )
            at_sb = sb.tile([128, 8, BL], F32, tag="atsb")
            nc.vector.tensor_copy(at_sb[:], at_ps[:])
            # PE-transpose [d, b] -> [b, d] per 128-chunk, then one dense DMA
            at2_sb = sb.tile([BL, 8, 128], F32, tag="at2sb")
            for dc in range(8):
                tpa = ps2.tile([BL, 128], F32, tag="tpa")
                nc.tensor.matmul(
                    tpa[:], at_sb[:, dc, :], idn_sb[:, :], is_transpose=True
                )
                nc.vector.tensor_copy(at2_sb[:, dc, :], tpa[:])
            at_in = dram.tile([BL, 1024], F32, tag="at_in")
            at_out = dram.tile([B, 1024], F32, tag="at_out", addr_space="Shared")
            nc.sync.dma_start(at_in[:].rearrange("b (dc p) -> b dc p", dc=8), at2_sb[:])
            nc.gpsimd.collective_compute(
                "AllGather",
                ALU.bypass,
                ins=[at_in.opt()],
                outs=[at_out.opt()],
                replica_groups=RG,
            )

            # ------------- x^T: transpose attn_full, concat input^T -------------
            af_sb = sb.tile([B, 1024], F32, tag="af")
            nc.sync.dma_start(af_sb[:], at_out[:])
            xT_sb = sb.tile([128, 8, B], F32, tag="xT")
            for c in range(8):
                tp = ps.tile([128, B], F32, tag="tp")
                nc.tensor.matmul(
                    tp[:],
                    af_sb[:, c * 128 : (c + 1) * 128],
                    idn_sb[:B, :B],
                    is_transpose=True,
                )
                nc.vector.tensor_copy(xT_sb[:, c, :], tp[:])

            # ------------- LSTM layer 0 (gates for all B, local hidden) ---------
            g0_ps = ps.tile([B, 4 * JL], F32, tag="g")
            nmm = 16
            k = 0
            for c in range(8):
                nc.tensor.matmul(
                    g0_ps[:], xT_sb[:, c, :], w0T_sb[:, c, :],
                    start=(k == 0), stop=(k == nmm - 1),
                )
                k += 1
            for c in range(4):
                nc.tensor.matmul(
                    g0_ps[:], inT_sb[:, c, :], w0T_sb[:, 8 + c, :],
                    start=(k == 0), stop=(k == nmm - 1),
                )
                k += 1
            for c in range(4):
                nc.tensor.matmul(
                    g0_ps[:], h0T0_sb[:, c, :], whh0T_sb[:, c, :],
                    start=(k == 0), stop=(k == nmm - 1),
                )
                k += 1

            def lstm_cell(g_ps, b_sb, c0_sb, c_out, h_out, name):
                g_sb = sb.tile([B, 4 * JL], F32, tag=f"g{name}")
                nc.vector.tensor_add(g_sb[:], g_ps[:], b_sb[:])
                act = sb.tile([B, 4, JL], F32, tag=f"act{name}")
                nc.scalar.activation(act[:, 0, :], g_sb[:, 0:64], AFT.Sigmoid)
                nc.scalar.activation(act[:, 1, :], g_sb[:, 64:128], AFT.Sigmoid)
                nc.scalar.activation(act[:, 2, :], g_sb[:, 128:192], AFT.Tanh)
                nc.scalar.activation(act[:, 3, :], g_sb[:, 192:256], AFT.Sigmoid)
                t1 = sb.tile([B, JL], F32, tag=f"t1{name}")
                nc.vector.tensor_mul(t1[:], act[:, 1, :], c0_sb[:])
                t2 = sb.tile([B, JL], F32, tag=f"t2{name}")
                nc.vector.tensor_mul(t2[:], act[:, 0, :], act[:, 2, :])
                c_sb = sb.tile([B, JL], F32, tag=f"c{name}")
                nc.vector.tensor_add(c_sb[:], t1[:], t2[:])
                nc.sync.dma_start(c_out[:], c_sb[:])
                tc1 = sb.tile([B, JL], F32, tag=f"tc{name}")
                nc.scalar.activation(tc1[:], c_sb[:], AFT.Tanh)
                h_sb = sb.tile([B, JL], F32, tag=f"h{name}")
                nc.vector.tensor_mul(h_sb[:], act[:, 3, :], tc1[:])
                nc.sync.dma_start(h_out[:], h_sb[:])
                return h_sb

            h1_sb = lstm_cell(g0_ps, b0_sb, c0l0_sb, c1_o, h1_o, "0")

            # transpose h1_l -> [JL, B], AllGather -> h1T_full [H, B]
            h1t_ps = ps.tile([JL, B], F32, tag="ht")
            nc.tensor.matmul(h1t_ps[:], h1_sb[:], idn_sb[:B, :B], is_transpose=True)
            h1t_sb = sb.tile([JL, B], F32, tag="h1ts")
            nc.vector.tensor_copy(h1t_sb[:], h1t_ps[:])
            h1_in = dram.tile([JL, B], F32, tag="h1_in")
            h1_out = dram.tile([H, B], F32, tag="h1_out", addr_space="Shared")
            nc.sync.dma_start(h1_in[:], h1t_sb[:])
            nc.gpsimd.collective_compute(
                "AllGather", ALU.bypass,
                ins=[h1_in.opt()], outs=[h1_out.opt()], replica_groups=RG,
            )
            h1T_sb = sb.tile([128, 4, B], F32, tag="h1T")
            nc.sync.dma_start(
                h1T_sb[:], h1_out[:].rearrange("(k p) b -> p k b", k=4)
            )

            # ------------- LSTM layer 1 -------------
            g1_ps = ps.tile([B, 4 * JL], F32, tag="g")
            nmm = 8
            k = 0
            for c in range(4):
                nc.tensor.matmul(
                    g1_ps[:], h1T_sb[:, c, :], w1T_sb[:, c, :],
                    start=(k == 0), stop=(k == nmm - 1),
                )
                k += 1
            for c in range(4):
                nc.tensor.matmul(
                    g1_ps[:], h0T1_sb[:, c, :], whh1T_sb[:, c, :],
                    start=(k == 0), stop=(k == nmm - 1),
                )
                k += 1

            h2_sb = lstm_cell(g1_ps, b1_sb, c0l1_sb, c2_o, h2_o, "1")

            h2t_ps = ps.tile([JL, B], F32, tag="ht")
            nc.tensor.matmul(h2t_ps[:], h2_sb[:], idn_sb[:B, :B], is_transpose=True)
            h2t_sb = sb.tile([JL, B], F32, tag="h2ts")
            nc.vector.tensor_copy(h2t_sb[:], h2t_ps[:])
            h2_in = dram.tile([JL, B], F32, tag="h2_in")
            h2_out = dram.tile([H, B], F32, tag="h2_out", addr_space="Shared")
            nc.sync.dma_start(h2_in[:], h2t_sb[:])
            nc.gpsimd.collective_compute(
                "AllGather", ALU.bypass,
                ins=[h2_in.opt()], outs=[h2_out.opt()], replica_groups=RG,
            )
            h2T_sb = sb.tile([128, 4, B], BF16, tag="h2T")
            nc.gpsimd.dma_start(
                h2T_sb[:], h2_out[:].rearrange("(k p) b -> p k b", k=4)
            )

            # ------------- vocab projection (bf16, local 4000 cols) -------------
            NB = 8
            CH = VL // NB  # 500 (one matmul must fit one PSUM bank)
            for nb in range(NB):
                pp = ps2.tile([B, CH], F32, tag="pp")
                for k in range(4):
                    nc.tensor.matmul(
                        pp[:],
                        h2T_sb[:, k, :],
                        owT_sb[:, k, nb * CH : (nb + 1) * CH],
                        start=(k == 0),
                        stop=(k == 3),
                    )
                pr = sb.tile([B, CH], F32, tag="pr")
                nc.vector.tensor_add(pr[:], pp[:], ob_sb[:, nb * CH : (nb + 1) * CH])
                nc.sync.dma_start(pred_o[:, nb * CH : (nb + 1) * CH], pr[:])

    if not nc.is_finalized():
        nc.finalize()
    return nc


def _prep_inputs(input, h0, c0, encoder_outputs, attn_W, attn_b,
                 w_ih0, w_hh0, b_ih0, b_hh0, w_ih1, w_hh1, b_ih1, b_hh1,
                 out_W, out_b):
    import ml_dtypes

    f = np.float32
    we = np.ascontiguousarray(attn_W[0, H:]).astype(f)           # [1024]
    web = np.ascontiguousarray(np.broadcast_to(we, (128, 1024))).astype(ml_dtypes.bfloat16)
    idn = np.eye(128, dtype=f)
    inT = np.ascontiguousarray(input.T).reshape(4, 128, B).astype(f)
    h0T0 = np.ascontiguousarray(h0[0].T).reshape(4, 128, B).astype(f)
    h0T1 = np.ascontiguousarray(h0[1].T).reshape(4, 128, B).astype(f)

    def gate_shard(w, jsl, kdim):
        # rows grouped [i|f|g|o] x JL for the shard, then transposed -> [K, 4*JL]
        blk = w.reshape(4, H, kdim)[:, jsl, :]          # [4, JL, K]
        t = np.ascontiguousarray(np.transpose(blk, (2, 0, 1)))  # [K, 4, JL]
        return t.reshape(kdim // 128, 128, 4 * JL).astype(f)

    def bias_shard(bi, bh, jsl):
        bb = (bi + bh).reshape(4, H)[:, jsl].reshape(4 * JL)
        return np.ascontiguousarray(np.broadcast_to(bb, (B, 4 * JL))).astype(f)

    in_maps = []
    for r in range(NCORES):
        bsl = slice(r * BL, (r + 1) * BL)
        jsl = slice(r * JL, (r + 1) * JL)
        vsl = slice(r * VL, (r + 1) * VL)
        enc_r = np.ascontiguousarray(encoder_outputs[:, bsl, :]).reshape(
            2, 128, BL * 1024).astype(ml_dtypes.bfloat16)
        sel = np.zeros((BL, 64), dtype=f)
        for i in range(BL):
            sel[i, r * BL + i] = 1.0
        sel_b = np.ascontiguousarray(np.broadcast_to(sel, (128, BL, 64)))
        owT = np.ascontiguousarray(out_W[vsl].T).reshape(4, 128, VL)
        ob = np.ascontiguousarray(np.broadcast_to(out_b[vsl], (B, VL)))
        in_maps.append({
            "enc": enc_r,
            "web": web,
            "sel": sel_b,
            "idn": idn,
            "inT": inT,
            "h0T0": h0T0,
            "h0T1": h0T1,
            "c0l0": np.ascontiguousarray(c0[0][:, jsl]).astype(f),
            "c0l1": np.ascontiguousarray(c0[1][:, jsl]).astype(f),
            "w0T": gate_shard(w_ih0, jsl, 3 * H),
            "whh0T": gate_shard(w_hh0, jsl, H),
            "b0": bias_shard(b_ih0, b_hh0, jsl),
            "w1T": gate_shard(w_ih1, jsl, H),
            "whh1T": gate_shard(w_hh1, jsl, H),
            "b1": bias_shard(b_ih1, b_hh1, jsl),
            "owT": owT.astype(ml_dtypes.bfloat16),
            "ob": ob.astype(ml_dtypes.bfloat16),
        })
    return in_maps


def kernel(**inputs):
    global _NC, LAST_RESULTS
    inputs = {k: np.asarray(v) for k, v in inputs.items()}
    if _NC is None:
        _NC = _build()
    in_maps = _prep_inputs(**inputs)
    res = run_bass_kernel_spmd(
        _NC, in_maps, core_ids=list(range(NCORES)),
        trace=bool(os.environ.get("BASS_TRACE")),
    )
    LAST_RESULTS = res
    out = res.results
    h1 = np.concatenate([out[r]["h1_l"] for r in range(NCORES)], axis=1)
    c1 = np.concatenate([out[r]["c1_l"] for r in range(NCORES)], axis=1)
    h2 = np.concatenate([out[r]["h2_l"] for r in range(NCORES)], axis=1)
    c2 = np.concatenate([out[r]["c2_l"] for r in range(NCORES)], axis=1)
    pred = np.concatenate([out[r]["pred_l"] for r in range(NCORES)], axis=1)
    output = h2[None].astype(np.float32)
    h_new = np.stack([h1, h2]).astype(np.float32)
    c_new = np.stack([c1, c2]).astype(np.float32)
    return output, h_new, c_new, pred.astype(np.float32)


# revision 17
# speedup vs baseline: 1.5842x; 1.0147x over previous
"""Distributed Trainium2 kernel for nn_AttnDecoderLSTM.

Sharding (8 cores):
  - Attention: data-parallel over batch B=64 (8 batches/core). Each core
    computes its score columns from its encoder_outputs shard, AllGathers the
    tiny [64,256] score matrix, replicates the (cheap, quirky reshape-)softmax,
    then computes attention sums for its local batches via PE matvecs.
  - LSTM: tensor-parallel over hidden (64 hidden/core, all 4 gates).  x and h
    are AllGathered in transposed [feature, batch] layout, which is exactly the
    lhsT layout the next matmul needs.
  - Output projection: tensor-parallel over vocab (4000/core), in bf16.

Math note: the reference adds h_top @ Wh.T + attn_b to the scores, but both
terms are constant within each 64-wide softmax chunk (the chunk spans one
batch row's consecutive columns), so they cancel in the softmax exactly and
are omitted.
"""

import os
import sys

import numpy as np

sys.path.insert(0, "/opt/trn_rl_repo")

import concourse.bass as bass
import concourse.bacc as bacc
import concourse.mybir as mybir
from concourse import tile
from concourse.bass_utils import run_bass_kernel_spmd

F32 = mybir.dt.float32
BF16 = mybir.dt.bfloat16
AX = mybir.AxisListType.X
AFT = mybir.ActivationFunctionType
ALU = mybir.AluOpType

NCORES = 8
B, H, S, V = 64, 512, 256, 32000
BL = B // NCORES      # 8 batches per core
JL = H // NCORES      # 64 hidden per core
VL = V // NCORES      # 4000 vocab per core
RG = [list(range(NCORES))]

_NC = None
LAST_RESULTS = None


def _build():
    nc = bacc.Bacc()

    def pi(name, shape, dt=F32):
        return nc.declare_dram_parameter(name, list(shape), dt, isOutput=False)

    def po(name, shape, dt=F32):
        return nc.declare_dram_parameter(name, list(shape), dt, isOutput=True)

    enc_p = pi("enc", [2, 128, BL * 1024], BF16)  # [s-tile, s, (b d)]
    web_p = pi("web", [128, 1024], BF16)         # We bcast over partitions
    sel_p = pi("sel", [128, BL, 64])             # one-hot b_local selectors
    idn_p = pi("idn", [128, 128])                # identity for PE transpose
    inT_p = pi("inT", [4, 128, B], BF16)               # input.T  (k-tiles)
    h0T0_p = pi("h0T0", [4, 128, B], BF16)
    h0T1_p = pi("h0T1", [4, 128, B], BF16)
    c0l0_p = pi("c0l0", [B, JL])
    c0l1_p = pi("c0l1", [B, JL])
    w0T_p = pi("w0T", [12, 128, 4 * JL], BF16)         # w_ih0 shard, transposed
    whh0T_p = pi("whh0T", [4, 128, 4 * JL], BF16)
    b0_p = pi("b0", [B, 4 * JL])
    w1T_p = pi("w1T", [4, 128, 4 * JL], BF16)
    whh1T_p = pi("whh1T", [4, 128, 4 * JL], BF16)
    b1_p = pi("b1", [B, 4 * JL])
    owT_p = pi("owT", [4, 128, VL], BF16)        # out_W shard, transposed, bf16
    ob_p = pi("ob", [B, VL], BF16)

    h1_o = po("h1_l", [B, JL])
    c1_o = po("c1_l", [B, JL])
    h2_o = po("h2_l", [B, JL])
    c2_o = po("c2_l", [B, JL])
    pred_o = po("pred_l", [B, VL])

    with tile.TileContext(nc) as tc:
        with (
            tc.tile_pool(name="sb", bufs=1) as sb,
            tc.tile_pool(name="ps", bufs=1, space="PSUM") as ps,
            tc.tile_pool(name="ps2", bufs=2, space="PSUM") as ps2,
            tc.tile_pool(name="dram", bufs=1, space="DRAM") as dram,
        ):
            # warm up the collective path while input DMAs stream
            dum_in = dram.tile([BL, 8], F32, tag="dum_in")
            dum_out = dram.tile([B, 8], F32, tag="dum_out", addr_space="Shared")
            nc.gpsimd.collective_compute(
                "AllGather", ALU.bypass,
                ins=[dum_in.opt()], outs=[dum_out.opt()], replica_groups=RG,
            )

            # ---------------- loads ----------------
            enc_sb = []
            for t in range(2):
                e = sb.tile([128, BL, 1024], BF16, tag=f"enc{t}")
                nc.sync.dma_start(e[:], enc_p[t].rearrange("p (b d) -> p b d", b=BL))
                enc_sb.append(e)

            web_sb = sb.tile([128, 1024], BF16, tag="web")
            nc.sync.dma_start(web_sb[:], web_p[:])
            sel_sb = sb.tile([128, BL, 64], F32, tag="sel")
            nc.sync.dma_start(sel_sb[:], sel_p[:])
            idn_sb = sb.tile([128, 128], F32, tag="idn")
            nc.sync.dma_start(idn_sb[:], idn_p[:])

            inT_sb = sb.tile([128, 4, B], BF16, tag="inT")
            nc.sync.dma_start(inT_sb[:], inT_p[:].rearrange("k p b -> p k b"))
            h0T0_sb = sb.tile([128, 4, B], BF16, tag="h0T0")
            nc.sync.dma_start(h0T0_sb[:], h0T0_p[:].rearrange("k p b -> p k b"))
            h0T1_sb = sb.tile([128, 4, B], BF16, tag="h0T1")
            nc.sync.dma_start(h0T1_sb[:], h0T1_p[:].rearrange("k p b -> p k b"))
            c0l0_sb = sb.tile([B, JL], F32, tag="c0l0")
            nc.sync.dma_start(c0l0_sb[:], c0l0_p[:])
            c0l1_sb = sb.tile([B, JL], F32, tag="c0l1")
            nc.sync.dma_start(c0l1_sb[:], c0l1_p[:])

            w0T_sb = sb.tile([128, 12, 4 * JL], BF16, tag="w0T")
            nc.sync.dma_start(w0T_sb[:], w0T_p[:].rearrange("k p j -> p k j"))
            whh0T_sb = sb.tile([128, 4, 4 * JL], BF16, tag="whh0T")
            nc.sync.dma_start(whh0T_sb[:], whh0T_p[:].rearrange("k p j -> p k j"))
            b0_sb = sb.tile([B, 4 * JL], F32, tag="b0")
            nc.sync.dma_start(b0_sb[:], b0_p[:])
            w1T_sb = sb.tile([128, 4, 4 * JL], BF16, tag="w1T")
            nc.sync.dma_start(w1T_sb[:], w1T_p[:].rearrange("k p j -> p k j"))
            whh1T_sb = sb.tile([128, 4, 4 * JL], BF16, tag="whh1T")
            nc.sync.dma_start(whh1T_sb[:], whh1T_p[:].rearrange("k p j -> p k j"))
            b1_sb = sb.tile([B, 4 * JL], F32, tag="b1")
            nc.sync.dma_start(b1_sb[:], b1_p[:])

            owT_sb = sb.tile([128, 4, VL], BF16, tag="owT")
            for k in range(4):
                nc.sync.dma_start(owT_sb[:, k, :], owT_p[k])
            ob_sb = sb.tile([B, VL], BF16, tag="ob")
            nc.sync.dma_start(ob_sb[:], ob_p[:])

            # ------------- scores: es[s, b] = enc[s,b,:] . We -------------
            prod = sb.tile([128, 1024], BF16, tag="prod")
            es_sb = sb.tile([128, 2, BL], F32, tag="es")
            for t in range(2):
                for b in range(BL):
                    nc.vector.tensor_mul(prod[:], enc_sb[t][:, b, :], web_sb[:])
                    nc.vector.reduce_sum(
                        out=es_sb[:, t, b : b + 1], in_=prod[:], axis=AX
                    )

            sc_in = dram.tile([BL, 256], F32, tag="sc_in")
            sc_out = dram.tile([B, 256], F32, tag="sc_out", addr_space="Shared")
            # write scores b-major: element (p,t,b) -> addr b*256 + t*128 + p
            for t in range(2):
                nc.gpsimd.dma_start(
                    sc_in[:].rearrange("b (t p) -> p t b", t=2)[:, t, :],
                    es_sb[:, t, :],
                )
            nc.gpsimd.collective_compute(
                "AllGather",
                ALU.bypass,
                ins=[sc_in.opt()],
                outs=[sc_out.opt()],
                replica_groups=RG,
            )

            # ------------- softmax over 64-wide chunks -------------
            sc_sb = sb.tile([B, 4, 64], F32, tag="sc")
            nc.sync.dma_start(sc_sb[:], sc_out[:].rearrange("q (r b) -> q r b", r=4))
            m_sb = sb.tile([B, 4], F32, tag="m")
            for r in range(4):
                nc.vector.reduce_max(
                    out=m_sb[:, r : r + 1], in_=sc_sb[:, r, :], axis=AX
                )
            negm = sb.tile([B, 4], F32, tag="negm")
            nc.scalar.mul(negm[:], m_sb[:], -1.0)
            e_sb = sb.tile([B, 4, 64], F32, tag="e")
            d_sb = sb.tile([B, 4], F32, tag="d")
            for r in range(4):
                nc.scalar.activation(
                    e_sb[:, r, :],
                    sc_sb[:, r, :],
                    AFT.Exp,
                    bias=negm[:, r : r + 1],
                    accum_out=d_sb[:, r : r + 1],
                )
            rd_sb = sb.tile([B, 4], F32, tag="rd")
            nc.vector.reciprocal(rd_sb[:], d_sb[:])
            n_sb = sb.tile([B, 4, 64], F32, tag="n")
            for r in range(4):
                nc.vector.tensor_scalar_mul(
                    n_sb[:, r, :], e_sb[:, r, :], rd_sb[:, r : r + 1]
                )

            # remap n[q, r, b] -> w[s, b] with s = 4q + r  (via DRAM)
            wrem = dram.tile([256, B], F32, tag="wrem")
            nc.gpsimd.dma_start(
                wrem[:].rearrange("(q r) b -> q r b", r=4), n_sb[:]
            )
            w_sb = sb.tile([128, 2, B], F32, tag="wsb")
            nc.gpsimd.dma_start(
                w_sb[:], wrem[:].rearrange("(t p) b -> p t b", t=2)
            )

            # select local batch columns: wse[s, i] = sum_b w[s, b] sel[i, b]
            wse = sb.tile([128, 2, BL], F32, tag="wse")
            wprod = sb.tile([128, 64], F32, tag="wprod")
            for t in range(2):
                for i in range(BL):
                    nc.vector.tensor_mul(wprod[:], w_sb[:, t, :], sel_sb[:, i, :])
                    nc.vector.reduce_sum(
                        out=wse[:, t, i : i + 1], in_=wprod[:], axis=AX
                    )

            wse_bf = sb.tile([128, 2, BL], BF16, tag="wsebf")
            nc.vector.tensor_copy(wse_bf[:], wse[:])

            # ------------- attention sums (per local batch matvec) -------------
            # at_row[0, b, d] = sum_s w[s, b] * enc[s, b, d]   (M=1, N=512)
            at_row = sb.tile([1, BL, 1024], F32, tag="atrow")
            for b in range(BL):
                for nch in range(2):
                    pb = ps2.tile([1, 512], F32, tag="pb")
                    for t in range(2):
                        nc.tensor.matmul(
                            pb[0:1, :],
                            wse_bf[:, t, b : b + 1],
                            enc_sb[t][:, b, nch * 512 : (nch + 1) * 512],
                            start=(t == 0),
                            stop=(t == 1),
                        )
                    dst = at_row[0:1, b, nch * 512 : (nch + 1) * 512]
                    if (b * 2 + nch) % 2 == 0:
                        nc.vector.tensor_copy(dst, pb[0:1, :])
                    else:
                        nc.scalar.copy(dst, pb[0:1, :])
            at_in = dram.tile([BL, 1024], F32, tag="at_in")
            at_out = dram.tile([B, 1024], F32, tag="at_out", addr_space="Shared")
            nc.sync.dma_start(at_in[:], at_row[0:1, :, :])
            nc.gpsimd.collective_compute(
                "AllGather",
                ALU.bypass,
                ins=[at_in.opt()],
                outs=[at_out.opt()],
                replica_groups=RG,
            )

            # ------------- x^T: transpose attn_full, concat input^T -------------
            af_sb = sb.tile([B, 1024], F32, tag="af")
            nc.sync.dma_start(af_sb[:], at_out[:])
            xT_sb = sb.tile([128, 8, B], BF16, tag="xT")
            for c in range(8):
                tp = ps.tile([128, B], F32, tag="tp")
                nc.tensor.matmul(
                    tp[:],
                    af_sb[:, c * 128 : (c + 1) * 128],
                    idn_sb[:B, :B],
                    is_transpose=True,
                )
                nc.vector.tensor_copy(xT_sb[:, c, :], tp[:])

            # ------------- LSTM layer 0 (gates for all B, local hidden) ---------
            g0_ps = ps.tile([B, 4 * JL], F32, tag="g")
            nmm = 16
            k = 0
            for c in range(8):
                nc.tensor.matmul(
                    g0_ps[:], xT_sb[:, c, :], w0T_sb[:, c, :],
                    start=(k == 0), stop=(k == nmm - 1),
                )
                k += 1
            for c in range(4):
                nc.tensor.matmul(
                    g0_ps[:], inT_sb[:, c, :], w0T_sb[:, 8 + c, :],
                    start=(k == 0), stop=(k == nmm - 1),
                )
                k += 1
            for c in range(4):
                nc.tensor.matmul(
                    g0_ps[:], h0T0_sb[:, c, :], whh0T_sb[:, c, :],
                    start=(k == 0), stop=(k == nmm - 1),
                )
                k += 1

            def lstm_cell(g_ps, b_sb, c0_sb, c_out, h_out, name):
                g_sb = sb.tile([B, 4 * JL], F32, tag=f"g{name}")
                nc.vector.tensor_add(g_sb[:], g_ps[:], b_sb[:])
                act = sb.tile([B, 4, JL], F32, tag=f"act{name}")
                nc.scalar.activation(act[:, 0, :], g_sb[:, 0:64], AFT.Sigmoid)
                nc.scalar.activation(act[:, 1, :], g_sb[:, 64:128], AFT.Sigmoid)
                nc.scalar.activation(act[:, 2, :], g_sb[:, 128:192], AFT.Tanh)
                nc.scalar.activation(act[:, 3, :], g_sb[:, 192:256], AFT.Sigmoid)
                t1 = sb.tile([B, JL], F32, tag=f"t1{name}")
                nc.vector.tensor_mul(t1[:], act[:, 1, :], c0_sb[:])
                t2 = sb.tile([B, JL], F32, tag=f"t2{name}")
                nc.vector.tensor_mul(t2[:], act[:, 0, :], act[:, 2, :])
                c_sb = sb.tile([B, JL], F32, tag=f"c{name}")
                nc.vector.tensor_add(c_sb[:], t1[:], t2[:])
                nc.sync.dma_start(c_out[:], c_sb[:])
                tc1 = sb.tile([B, JL], F32, tag=f"tc{name}")
                nc.scalar.activation(tc1[:], c_sb[:], AFT.Tanh)
                h_sb = sb.tile([B, JL], F32, tag=f"h{name}")
                nc.vector.tensor_mul(h_sb[:], act[:, 3, :], tc1[:])
                nc.sync.dma_start(h_out[:], h_sb[:])
                return h_sb

            h1_sb = lstm_cell(g0_ps, b0_sb, c0l0_sb, c1_o, h1_o, "0")

            # transpose h1_l -> [JL, B], AllGather -> h1T_full [H, B]
            h1t_ps = ps.tile([JL, B], F32, tag="ht")
            nc.tensor.matmul(h1t_ps[:], h1_sb[:], idn_sb[:B, :B], is_transpose=True)
            h1t_sb = sb.tile([JL, B], F32, tag="h1ts")
            nc.vector.tensor_copy(h1t_sb[:], h1t_ps[:])
            h1_in = dram.tile([JL, B], F32, tag="h1_in")
            h1_out = dram.tile([H, B], F32, tag="h1_out", addr_space="Shared")
            nc.sync.dma_start(h1_in[:], h1t_sb[:])
            nc.gpsimd.collective_compute(
                "AllGather", ALU.bypass,
                ins=[h1_in.opt()], outs=[h1_out.opt()], replica_groups=RG,
            )
            h1T_sb = sb.tile([128, 4, B], BF16, tag="h1T")
            nc.gpsimd.dma_start(
                h1T_sb[:], h1_out[:].rearrange("(k p) b -> p k b", k=4)
            )

            # ------------- LSTM layer 1 -------------
            g1_ps = ps.tile([B, 4 * JL], F32, tag="g")
            nmm = 8
            k = 0
            for c in range(4):
                nc.tensor.matmul(
                    g1_ps[:], h1T_sb[:, c, :], w1T_sb[:, c, :],
                    start=(k == 0), stop=(k == nmm - 1),
                )
                k += 1
            for c in range(4):
                nc.tensor.matmul(
                    g1_ps[:], h0T1_sb[:, c, :], whh1T_sb[:, c, :],
                    start=(k == 0), stop=(k == nmm - 1),
                )
                k += 1

            h2_sb = lstm_cell(g1_ps, b1_sb, c0l1_sb, c2_o, h2_o, "1")

            h2t_ps = ps.tile([JL, B], F32, tag="ht")
            nc.tensor.matmul(h2t_ps[:], h2_sb[:], idn_sb[:B, :B], is_transpose=True)
            h2t_sb = sb.tile([JL, B], F32, tag="h2ts")
            nc.vector.tensor_copy(h2t_sb[:], h2t_ps[:])
            h2_in = dram.tile([JL, B], F32, tag="h2_in")
            h2_out = dram.tile([H, B], F32, tag="h2_out", addr_space="Shared")
            nc.sync.dma_start(h2_in[:], h2t_sb[:])
            nc.gpsimd.collective_compute(
                "AllGather", ALU.bypass,
                ins=[h2_in.opt()], outs=[h2_out.opt()], replica_groups=RG,
            )
            h2T_sb = sb.tile([128, 4, B], BF16, tag="h2T")
            nc.gpsimd.dma_start(
                h2T_sb[:], h2_out[:].rearrange("(k p) b -> p k b", k=4)
            )

            # ------------- vocab projection (bf16, local 4000 cols) -------------
            NB = 8
            CH = VL // NB  # 500 (one matmul must fit one PSUM bank)
            for nb in range(NB):
                pp = ps2.tile([B, CH], F32, tag="pp")
                for k in range(4):
                    nc.tensor.matmul(
                        pp[:],
                        h2T_sb[:, k, :],
                        owT_sb[:, k, nb * CH : (nb + 1) * CH],
                        start=(k == 0),
                        stop=(k == 3),
                    )
                pr = sb.tile([B, CH], F32, tag="pr")
                nc.vector.tensor_add(pr[:], pp[:], ob_sb[:, nb * CH : (nb + 1) * CH])
                nc.sync.dma_start(pred_o[:, nb * CH : (nb + 1) * CH], pr[:])

    if not nc.is_finalized():
        nc.finalize()
    return nc


def _prep_inputs(input, h0, c0, encoder_outputs, attn_W, attn_b,
                 w_ih0, w_hh0, b_ih0, b_hh0, w_ih1, w_hh1, b_ih1, b_hh1,
                 out_W, out_b):
    import ml_dtypes

    f = np.float32
    we = np.ascontiguousarray(attn_W[0, H:]).astype(f)           # [1024]
    web = np.ascontiguousarray(np.broadcast_to(we, (128, 1024))).astype(ml_dtypes.bfloat16)
    idn = np.eye(128, dtype=f)
    inT = np.ascontiguousarray(input.T).reshape(4, 128, B).astype(ml_dtypes.bfloat16)
    h0T0 = np.ascontiguousarray(h0[0].T).reshape(4, 128, B).astype(ml_dtypes.bfloat16)
    h0T1 = np.ascontiguousarray(h0[1].T).reshape(4, 128, B).astype(ml_dtypes.bfloat16)

    def gate_shard(w, jsl, kdim):
        # rows grouped [i|f|g|o] x JL for the shard, then transposed -> [K, 4*JL]
        blk = w.reshape(4, H, kdim)[:, jsl, :]          # [4, JL, K]
        t = np.ascontiguousarray(np.transpose(blk, (2, 0, 1)))  # [K, 4, JL]
        return t.reshape(kdim // 128, 128, 4 * JL).astype(ml_dtypes.bfloat16)

    def bias_shard(bi, bh, jsl):
        bb = (bi + bh).reshape(4, H)[:, jsl].reshape(4 * JL)
        return np.ascontiguousarray(np.broadcast_to(bb, (B, 4 * JL))).astype(f)

    in_maps = []
    for r in range(NCORES):
        bsl = slice(r * BL, (r + 1) * BL)
        jsl = slice(r * JL, (r + 1) * JL)
        vsl = slice(r * VL, (r + 1) * VL)
        enc_r = np.ascontiguousarray(encoder_outputs[:, bsl, :]).reshape(
            2, 128, BL * 1024).astype(ml_dtypes.bfloat16)
        sel = np.zeros((BL, 64), dtype=f)
        for i in range(BL):
            sel[i, r * BL + i] = 1.0
        sel_b = np.ascontiguousarray(np.broadcast_to(sel, (128, BL, 64)))
        owT = np.ascontiguousarray(out_W[vsl].T).reshape(4, 128, VL)
        ob = np.ascontiguousarray(np.broadcast_to(out_b[vsl], (B, VL)))
        in_maps.append({
            "enc": enc_r,
            "web": web,
            "sel": sel_b,
            "idn": idn,
            "inT": inT,
            "h0T0": h0T0,
            "h0T1": h0T1,
            "c0l0": np.ascontiguousarray(c0[0][:, jsl]).astype(f),
            "c0l1": np.ascontiguousarray(c0[1][:, jsl]).astype(f),
            "w0T": gate_shard(w_ih0, jsl, 3 * H),
            "whh0T": gate_shard(w_hh0, jsl, H),
            "b0": bias_shard(b_ih0, b_hh0, jsl),
            "w1T": gate_shard(w_ih1, jsl, H),
            "whh1T": gate_shard(w_hh1, jsl, H),
            "b1": bias_shard(b_ih1, b_hh1, jsl),
            "owT": owT.astype(ml_dtypes.bfloat16),
            "ob": ob.astype(ml_dtypes.bfloat16),
        })
    return in_maps


def kernel(**inputs):
    global _NC, LAST_RESULTS
    inputs = {k: np.asarray(v) for k, v in inputs.items()}
    if _NC is None:
        _NC = _build()
    in_maps = _prep_inputs(**inputs)
    res = run_bass_kernel_spmd(
        _NC, in_maps, core_ids=list(range(NCORES)),
        trace=bool(os.environ.get("BASS_TRACE")),
    )
    LAST_RESULTS = res
    out = res.results
    h1 = np.concatenate([out[r]["h1_l"] for r in range(NCORES)], axis=1)
    c1 = np.concatenate([out[r]["c1_l"] for r in range(NCORES)], axis=1)
    h2 = np.concatenate([out[r]["h2_l"] for r in range(NCORES)], axis=1)
    c2 = np.concatenate([out[r]["c2_l"] for r in range(NCORES)], axis=1)
    pred = np.concatenate([out[r]["pred_l"] for r in range(NCORES)], axis=1)
    output = h2[None].astype(np.float32)
    h_new = np.stack([h1, h2]).astype(np.float32)
    c_new = np.stack([c1, c2]).astype(np.float32)
    return output, h_new, c_new, pred.astype(np.float32)


# revision 19
# speedup vs baseline: 2.0595x; 1.3000x over previous
"""Distributed Trainium2 kernel for nn_AttnDecoderLSTM.

Sharding (8 cores):
  - Attention: data-parallel over batch B=64 (8 batches/core). Each core
    computes its score columns from its encoder_outputs shard, AllGathers the
    tiny [64,256] score matrix, replicates the (cheap, quirky reshape-)softmax,
    then computes attention sums for its local batches via PE matvecs.
  - LSTM: tensor-parallel over hidden (64 hidden/core, all 4 gates).  x and h
    are AllGathered in transposed [feature, batch] layout, which is exactly the
    lhsT layout the next matmul needs.
  - Output projection: tensor-parallel over vocab (4000/core), in bf16.

Math note: the reference adds h_top @ Wh.T + attn_b to the scores, but both
terms are constant within each 64-wide softmax chunk (the chunk spans one
batch row's consecutive columns), so they cancel in the softmax exactly and
are omitted.
"""

import os
import sys

import numpy as np

sys.path.insert(0, "/opt/trn_rl_repo")

import concourse.bass as bass
import concourse.bacc as bacc
import concourse.mybir as mybir
from concourse import tile
from concourse.bass_utils import run_bass_kernel_spmd

F32 = mybir.dt.float32
BF16 = mybir.dt.bfloat16
AX = mybir.AxisListType.X
AFT = mybir.ActivationFunctionType
ALU = mybir.AluOpType

NCORES = 8
B, H, S, V = 64, 512, 256, 32000
BL = B // NCORES      # 8 batches per core
JL = H // NCORES      # 64 hidden per core
VL = V // NCORES      # 4000 vocab per core
RG = [list(range(NCORES))]

_NC = None
LAST_RESULTS = None


def _build():
    nc = bacc.Bacc()

    def pi(name, shape, dt=F32):
        return nc.declare_dram_parameter(name, list(shape), dt, isOutput=False)

    def po(name, shape, dt=F32):
        return nc.declare_dram_parameter(name, list(shape), dt, isOutput=True)

    enc_p = pi("enc", [2, 128, BL * 1024], BF16)  # [s-tile, s, (b d)]
    web_p = pi("web", [128, 1024], BF16)         # We bcast over partitions
    sel_p = pi("sel", [128, BL, 64])             # one-hot b_local selectors
    idn_p = pi("idn", [128, 128])                # identity for PE transpose
    inT_p = pi("inT", [4, 128, B], BF16)               # input.T  (k-tiles)
    h0T0_p = pi("h0T0", [4, 128, B], BF16)
    h0T1_p = pi("h0T1", [4, 128, B], BF16)
    c0l0_p = pi("c0l0", [B, JL])
    c0l1_p = pi("c0l1", [B, JL])
    w0T_p = pi("w0T", [12, 128, 4 * JL], BF16)         # w_ih0 shard, transposed
    whh0T_p = pi("whh0T", [4, 128, 4 * JL], BF16)
    b0_p = pi("b0", [B, 4 * JL])
    w1T_p = pi("w1T", [4, 128, 4 * JL], BF16)
    whh1T_p = pi("whh1T", [4, 128, 4 * JL], BF16)
    b1_p = pi("b1", [B, 4 * JL])
    owT_p = pi("owT", [4, 128, VL], BF16)        # out_W shard, transposed, bf16
    ob_p = pi("ob", [B, VL], BF16)

    h1_o = po("h1_l", [B, JL])
    c1_o = po("c1_l", [B, JL])
    h2_o = po("h2_l", [B, JL])
    c2_o = po("c2_l", [B, JL])
    pred_o = po("pred_l", [B, VL])

    with tile.TileContext(nc) as tc:
        with (
            tc.tile_pool(name="sb", bufs=1) as sb,
            tc.tile_pool(name="ps", bufs=1, space="PSUM") as ps,
            tc.tile_pool(name="ps2", bufs=2, space="PSUM") as ps2,
            tc.tile_pool(name="sb3", bufs=3) as sb3,
            tc.tile_pool(name="dram", bufs=1, space="DRAM") as dram,
        ):
            # warm up the collective path while input DMAs stream
            dum_in = dram.tile([BL, 8], F32, tag="dum_in")
            dum_out = dram.tile([B, 8], F32, tag="dum_out", addr_space="Shared")
            nc.gpsimd.collective_compute(
                "AllGather", ALU.bypass,
                ins=[dum_in.opt()], outs=[dum_out.opt()], replica_groups=RG,
            )

            # preload activation LUTs (exp/sigmoid/tanh) during the barrier
            warm = sb.tile([1, 4], F32, tag="warm")
            nc.gpsimd.memset(warm[0:1, :], 0.0)
            for fn in (AFT.Exp, AFT.Sigmoid, AFT.Tanh):
                nc.scalar.activation(warm[0:1, :], warm[0:1, :], fn)

            # ---------------- loads ----------------
            enc_sb = []
            for t in range(2):
                e = sb.tile([128, BL, 1024], BF16, tag=f"enc{t}")
                nc.sync.dma_start(e[:], enc_p[t].rearrange("p (b d) -> p b d", b=BL))
                enc_sb.append(e)

            web_sb = sb.tile([128, 1024], BF16, tag="web")
            nc.sync.dma_start(web_sb[:], web_p[:])
            sel_sb = sb.tile([128, BL, 64], F32, tag="sel")
            nc.sync.dma_start(sel_sb[:], sel_p[:])
            idn_sb = sb.tile([128, 128], F32, tag="idn")
            nc.sync.dma_start(idn_sb[:], idn_p[:])

            inT_sb = sb.tile([128, 4, B], BF16, tag="inT")
            nc.sync.dma_start(inT_sb[:], inT_p[:].rearrange("k p b -> p k b"))
            h0T0_sb = sb.tile([128, 4, B], BF16, tag="h0T0")
            nc.sync.dma_start(h0T0_sb[:], h0T0_p[:].rearrange("k p b -> p k b"))
            h0T1_sb = sb.tile([128, 4, B], BF16, tag="h0T1")
            nc.sync.dma_start(h0T1_sb[:], h0T1_p[:].rearrange("k p b -> p k b"))
            c0l0_sb = sb.tile([B, JL], F32, tag="c0l0")
            nc.sync.dma_start(c0l0_sb[:], c0l0_p[:])
            c0l1_sb = sb.tile([B, JL], F32, tag="c0l1")
            nc.sync.dma_start(c0l1_sb[:], c0l1_p[:])

            w0T_sb = sb.tile([128, 12, 4 * JL], BF16, tag="w0T")
            nc.sync.dma_start(w0T_sb[:], w0T_p[:].rearrange("k p j -> p k j"))
            whh0T_sb = sb.tile([128, 4, 4 * JL], BF16, tag="whh0T")
            nc.sync.dma_start(whh0T_sb[:], whh0T_p[:].rearrange("k p j -> p k j"))
            b0_sb = sb.tile([B, 4 * JL], F32, tag="b0")
            nc.sync.dma_start(b0_sb[:], b0_p[:])
            w1T_sb = sb.tile([128, 4, 4 * JL], BF16, tag="w1T")
            nc.sync.dma_start(w1T_sb[:], w1T_p[:].rearrange("k p j -> p k j"))
            whh1T_sb = sb.tile([128, 4, 4 * JL], BF16, tag="whh1T")
            nc.sync.dma_start(whh1T_sb[:], whh1T_p[:].rearrange("k p j -> p k j"))
            b1_sb = sb.tile([B, 4 * JL], F32, tag="b1")
            nc.sync.dma_start(b1_sb[:], b1_p[:])

            owT_sb = sb.tile([128, 4, VL], BF16, tag="owT")
            for k in range(4):
                nc.sync.dma_start(owT_sb[:, k, :], owT_p[k])
            ob_sb = sb.tile([B, VL], BF16, tag="ob")
            nc.sync.dma_start(ob_sb[:], ob_p[:])

            # ------------- scores: es[s, b] = enc[s,b,:] . We -------------
            prod = sb.tile([128, 1024], BF16, tag="prod")
            es_sb = sb.tile([128, 2, BL], F32, tag="es")
            for t in range(2):
                for b in range(BL):
                    nc.vector.tensor_mul(prod[:], enc_sb[t][:, b, :], web_sb[:])
                    nc.vector.reduce_sum(
                        out=es_sb[:, t, b : b + 1], in_=prod[:], axis=AX
                    )

            sc_in = dram.tile([BL, 256], F32, tag="sc_in")
            sc_out = dram.tile([B, 256], F32, tag="sc_out", addr_space="Shared")
            # write scores b-major: element (p,t,b) -> addr b*256 + t*128 + p
            for t in range(2):
                nc.gpsimd.dma_start(
                    sc_in[:].rearrange("b (t p) -> p t b", t=2)[:, t, :],
                    es_sb[:, t, :],
                )
            nc.gpsimd.collective_compute(
                "AllGather",
                ALU.bypass,
                ins=[sc_in.opt()],
                outs=[sc_out.opt()],
                replica_groups=RG,
            )

            # ------------- softmax over 64-wide chunks, in s-layout -------------
            # Flat sc_out IS weights_t [256 s, 64] (q*256 + r*64 + b = s*64 + b
            # with s = 4q + r).  Load as [p, t, b] with s = t*128 + p; the
            # softmax chunk is then just the free axis.  Scores are O(1), so
            # max-subtraction is skipped (fp32 exp is exact enough).
            sc2 = sb.tile([128, 2, 64], F32, tag="sc2")
            sc2_src = sc_out[:].rearrange(
                "(t qq) (r b) -> (qq r) t b", t=2, r=4
            )
            for t in range(2):
                nc.sync.dma_start(sc2[:, t, :], sc2_src[:, t, :])
            e2 = sb.tile([128, 2, 64], F32, tag="e2")
            d2 = sb.tile([128, 2], F32, tag="d2")
            for t in range(2):
                nc.scalar.activation(
                    e2[:, t, :], sc2[:, t, :], AFT.Exp,
                    accum_out=d2[:, t : t + 1],
                )
            rd2 = sb.tile([128, 2], F32, tag="rd2")
            nc.vector.reciprocal(rd2[:], d2[:])
            w_sb = sb.tile([128, 2, B], F32, tag="wsb")
            for t in range(2):
                nc.vector.tensor_scalar_mul(
                    w_sb[:, t, :], e2[:, t, :], rd2[:, t : t + 1]
                )

            # select local batch columns: wse[s, i] = sum_b w[s, b] sel[i, b]
            wse = sb.tile([128, 2, BL], F32, tag="wse")
            wprod = sb.tile([128, 64], F32, tag="wprod")
            for t in range(2):
                for i in range(BL):
                    nc.vector.tensor_mul(wprod[:], w_sb[:, t, :], sel_sb[:, i, :])
                    nc.vector.reduce_sum(
                        out=wse[:, t, i : i + 1], in_=wprod[:], axis=AX
                    )

            wse_bf = sb.tile([128, 2, BL], BF16, tag="wsebf")
            nc.vector.tensor_copy(wse_bf[:], wse[:])

            # ------------- attention sums (per local batch matvec) -------------
            # at_row[0, b, d] = sum_s w[s, b] * enc[s, b, d]   (M=1, N=512)
            at_row = sb.tile([1, BL, 1024], F32, tag="atrow")
            for b in range(BL):
                for nch in range(2):
                    pb = ps2.tile([1, 512], F32, tag="pb")
                    for t in range(2):
                        nc.tensor.matmul(
                            pb[0:1, :],
                            wse_bf[:, t, b : b + 1],
                            enc_sb[t][:, b, nch * 512 : (nch + 1) * 512],
                            start=(t == 0),
                            stop=(t == 1),
                        )
                    dst = at_row[0:1, b, nch * 512 : (nch + 1) * 512]
                    if (b * 2 + nch) % 2 == 0:
                        nc.vector.tensor_copy(dst, pb[0:1, :])
                    else:
                        nc.scalar.copy(dst, pb[0:1, :])
            at_in = dram.tile([BL, 1024], F32, tag="at_in")
            at_out = dram.tile([B, 1024], F32, tag="at_out", addr_space="Shared")
            nc.sync.dma_start(at_in[:], at_row[0:1, :, :])
            nc.gpsimd.collective_compute(
                "AllGather",
                ALU.bypass,
                ins=[at_in.opt()],
                outs=[at_out.opt()],
                replica_groups=RG,
            )

            # ------------- x^T: transpose attn_full, concat input^T -------------
            af_sb = sb.tile([B, 1024], F32, tag="af")
            nc.sync.dma_start(af_sb[:], at_out[:])
            xT_sb = sb.tile([128, 8, B], BF16, tag="xT")
            for c in range(8):
                tp = ps.tile([128, B], F32, tag="tp")
                nc.tensor.matmul(
                    tp[:],
                    af_sb[:, c * 128 : (c + 1) * 128],
                    idn_sb[:B, :B],
                    is_transpose=True,
                )
                nc.vector.tensor_copy(xT_sb[:, c, :], tp[:])

            # ------------- LSTM layer 0 (gates for all B, local hidden) ---------
            g0_ps = ps.tile([B, 4 * JL], F32, tag="g")
            nmm = 16
            k = 0
            for c in range(4):
                nc.tensor.matmul(
                    g0_ps[:], inT_sb[:, c, :], w0T_sb[:, 8 + c, :],
                    start=(k == 0), stop=(k == nmm - 1),
                )
                k += 1
            for c in range(4):
                nc.tensor.matmul(
                    g0_ps[:], h0T0_sb[:, c, :], whh0T_sb[:, c, :],
                    start=(k == 0), stop=(k == nmm - 1),
                )
                k += 1
            for c in range(8):
                nc.tensor.matmul(
                    g0_ps[:], xT_sb[:, c, :], w0T_sb[:, c, :],
                    start=(k == 0), stop=(k == nmm - 1),
                )
                k += 1

            def lstm_cell(g_ps, b_sb, c0_sb, c_out, h_out, name):
                # gate column order is [i | f | o | g] (host-arranged)
                g_sb = sb.tile([B, 4 * JL], F32, tag=f"g{name}")
                nc.vector.tensor_add(g_sb[:], g_ps[:], b_sb[:])
                act = sb.tile([B, 4, JL], F32, tag=f"act{name}")
                nc.scalar.activation(
                    act[:, 0:3, :].rearrange("p a j -> p (a j)"),
                    g_sb[:, 0:192], AFT.Sigmoid,
                )
                nc.scalar.activation(act[:, 3, :], g_sb[:, 192:256], AFT.Tanh)
                t1 = sb.tile([B, JL], F32, tag=f"t1{name}")
                nc.vector.tensor_mul(t1[:], act[:, 1, :], c0_sb[:])
                t2 = sb.tile([B, JL], F32, tag=f"t2{name}")
                nc.vector.tensor_mul(t2[:], act[:, 0, :], act[:, 3, :])
                c_sb = sb.tile([B, JL], F32, tag=f"c{name}")
                nc.vector.tensor_add(c_sb[:], t1[:], t2[:])
                nc.sync.dma_start(c_out[:], c_sb[:])
                tc1 = sb.tile([B, JL], F32, tag=f"tc{name}")
                nc.scalar.activation(tc1[:], c_sb[:], AFT.Tanh)
                h_sb = sb.tile([B, JL], F32, tag=f"h{name}")
                nc.vector.tensor_mul(h_sb[:], act[:, 2, :], tc1[:])
                nc.sync.dma_start(h_out[:], h_sb[:])
                return h_sb

            h1_sb = lstm_cell(g0_ps, b0_sb, c0l0_sb, c1_o, h1_o, "0")

            # transpose h1_l -> [JL, B], AllGather -> h1T_full [H, B]
            h1t_ps = ps.tile([JL, B], F32, tag="ht")
            nc.tensor.matmul(h1t_ps[:], h1_sb[:], idn_sb[:B, :B], is_transpose=True)
            h1t_sb = sb.tile([JL, B], F32, tag="h1ts")
            nc.vector.tensor_copy(h1t_sb[:], h1t_ps[:])
            h1_in = dram.tile([JL, B], F32, tag="h1_in")
            h1_out = dram.tile([H, B], F32, tag="h1_out", addr_space="Shared")
            nc.sync.dma_start(h1_in[:], h1t_sb[:])
            nc.gpsimd.collective_compute(
                "AllGather", ALU.bypass,
                ins=[h1_in.opt()], outs=[h1_out.opt()], replica_groups=RG,
            )
            h1T_sb = sb.tile([128, 4, B], BF16, tag="h1T")
            nc.gpsimd.dma_start(
                h1T_sb[:], h1_out[:].rearrange("(k p) b -> p k b", k=4)
            )

            # ------------- LSTM layer 1 -------------
            g1_ps = ps.tile([B, 4 * JL], F32, tag="g")
            nmm = 8
            k = 0
            for c in range(4):
                nc.tensor.matmul(
                    g1_ps[:], h0T1_sb[:, c, :], whh1T_sb[:, c, :],
                    start=(k == 0), stop=(k == nmm - 1),
                )
                k += 1
            for c in range(4):
                nc.tensor.matmul(
                    g1_ps[:], h1T_sb[:, c, :], w1T_sb[:, c, :],
                    start=(k == 0), stop=(k == nmm - 1),
                )
                k += 1

            h2_sb = lstm_cell(g1_ps, b1_sb, c0l1_sb, c2_o, h2_o, "1")

            h2t_ps = ps.tile([JL, B], F32, tag="ht")
            nc.tensor.matmul(h2t_ps[:], h2_sb[:], idn_sb[:B, :B], is_transpose=True)
            h2t_sb = sb.tile([JL, B], F32, tag="h2ts")
            nc.vector.tensor_copy(h2t_sb[:], h2t_ps[:])
            h2_in = dram.tile([JL, B], F32, tag="h2_in")
            h2_out = dram.tile([H, B], F32, tag="h2_out", addr_space="Shared")
            nc.sync.dma_start(h2_in[:], h2t_sb[:])
            nc.gpsimd.collective_compute(
                "AllGather", ALU.bypass,
                ins=[h2_in.opt()], outs=[h2_out.opt()], replica_groups=RG,
            )
            h2T_sb = sb.tile([128, 4, B], BF16, tag="h2T")
            nc.gpsimd.dma_start(
                h2T_sb[:], h2_out[:].rearrange("(k p) b -> p k b", k=4)
            )

            # ------------- vocab projection (bf16, local 4000 cols) -------------
            NB = 8
            CH = VL // NB  # 500 (one matmul must fit one PSUM bank)
            for nb in range(NB):
                pp = ps2.tile([B, CH], F32, tag="pp")
                for k in range(4):
                    nc.tensor.matmul(
                        pp[:],
                        h2T_sb[:, k, :],
                        owT_sb[:, k, nb * CH : (nb + 1) * CH],
                        start=(k == 0),
                        stop=(k == 3),
                    )
                pr = sb3.tile([B, CH], F32, tag="pr")
                nc.vector.tensor_add(pr[:], pp[:], ob_sb[:, nb * CH : (nb + 1) * CH])
                nc.sync.dma_start(pred_o[:, nb * CH : (nb + 1) * CH], pr[:])

    if not nc.is_finalized():
        nc.finalize()
    return nc


def _prep_inputs(input, h0, c0, encoder_outputs, attn_W, attn_b,
                 w_ih0, w_hh0, b_ih0, b_hh0, w_ih1, w_hh1, b_ih1, b_hh1,
                 out_W, out_b):
    import ml_dtypes

    f = np.float32
    we = np.ascontiguousarray(attn_W[0, H:]).astype(f)           # [1024]
    web = np.ascontiguousarray(np.broadcast_to(we, (128, 1024))).astype(ml_dtypes.bfloat16)
    idn = np.eye(128, dtype=f)
    inT = np.ascontiguousarray(input.T).reshape(4, 128, B).astype(ml_dtypes.bfloat16)
    h0T0 = np.ascontiguousarray(h0[0].T).reshape(4, 128, B).astype(ml_dtypes.bfloat16)
    h0T1 = np.ascontiguousarray(h0[1].T).reshape(4, 128, B).astype(ml_dtypes.bfloat16)

    def gate_shard(w, jsl, kdim):
        # rows grouped [i|f|o|g] x JL for the shard, then transposed -> [K, 4*JL]
        blk = w.reshape(4, H, kdim)[[0, 1, 3, 2]][:, jsl, :]  # [4, JL, K]
        t = np.ascontiguousarray(np.transpose(blk, (2, 0, 1)))  # [K, 4, JL]
        return t.reshape(kdim // 128, 128, 4 * JL).astype(ml_dtypes.bfloat16)

    def bias_shard(bi, bh, jsl):
        bb = (bi + bh).reshape(4, H)[[0, 1, 3, 2]][:, jsl].reshape(4 * JL)
        return np.ascontiguousarray(np.broadcast_to(bb, (B, 4 * JL))).astype(f)

    in_maps = []
    for r in range(NCORES):
        bsl = slice(r * BL, (r + 1) * BL)
        jsl = slice(r * JL, (r + 1) * JL)
        vsl = slice(r * VL, (r + 1) * VL)
        enc_r = np.ascontiguousarray(encoder_outputs[:, bsl, :]).reshape(
            2, 128, BL * 1024).astype(ml_dtypes.bfloat16)
        sel = np.zeros((BL, 64), dtype=f)
        for i in range(BL):
            sel[i, r * BL + i] = 1.0
        sel_b = np.ascontiguousarray(np.broadcast_to(sel, (128, BL, 64)))
        owT = np.ascontiguousarray(out_W[vsl].T).reshape(4, 128, VL)
        ob = np.ascontiguousarray(np.broadcast_to(out_b[vsl], (B, VL)))
        in_maps.append({
            "enc": enc_r,
            "web": web,
            "sel": sel_b,
            "idn": idn,
            "inT": inT,
            "h0T0": h0T0,
            "h0T1": h0T1,
            "c0l0": np.ascontiguousarray(c0[0][:, jsl]).astype(f),
            "c0l1": np.ascontiguousarray(c0[1][:, jsl]).astype(f),
            "w0T": gate_shard(w_ih0, jsl, 3 * H),
            "whh0T": gate_shard(w_hh0, jsl, H),
            "b0": bias_shard(b_ih0, b_hh0, jsl),
            "w1T": gate_shard(w_ih1, jsl, H),
            "whh1T": gate_shard(w_hh1, jsl, H),
            "b1": bias_shard(b_ih1, b_hh1, jsl),
            "owT": owT.astype(ml_dtypes.bfloat16),
            "ob": ob.astype(ml_dtypes.bfloat16),
        })
    return in_maps


def kernel(**inputs):
    global _NC, LAST_RESULTS
    inputs = {k: np.asarray(v) for k, v in inputs.items()}
    if _NC is None:
        _NC = _build()
    in_maps = _prep_inputs(**inputs)
    res = run_bass_kernel_spmd(
        _NC, in_maps, core_ids=list(range(NCORES)),
        trace=bool(os.environ.get("BASS_TRACE")),
    )
    LAST_RESULTS = res
    out = res.results
    h1 = np.concatenate([out[r]["h1_l"] for r in range(NCORES)], axis=1)
    c1 = np.concatenate([out[r]["c1_l"] for r in range(NCORES)], axis=1)
    h2 = np.concatenate([out[r]["h2_l"] for r in range(NCORES)], axis=1)
    c2 = np.concatenate([out[r]["c2_l"] for r in range(NCORES)], axis=1)
    pred = np.concatenate([out[r]["pred_l"] for r in range(NCORES)], axis=1)
    output = h2[None].astype(np.float32)
    h_new = np.stack([h1, h2]).astype(np.float32)
    c_new = np.stack([c1, c2]).astype(np.float32)
    return output, h_new, c_new, pred.astype(np.float32)
